# Initial kernel scaffold
#
"""TRN2 Bass kernel for nn_Attention_87308095193383.

Sharding: 8 cores = (batch b in 0..3) x (query-half h in 0..1).
Each core computes, for its batch:
  - conv1/conv2 + GroupNorm fully (stats need full N),
  - pe_attn^T slice [m=2048, n=1024] for its query half,
  - k,v fully; q for its half,
  - attention + proj for its half -> outT [512, 1024].
Host permutes the N columns per core so "my queries" are always columns
0:1024 of the device program (SPMD: one program, per-core data).

All matmuls use float32r (1 cycle/row on TRN2 for free-dim >= 256,
~1.5e-4 rel err). Softmax skips max-subtraction (logits are O(1) here:
|scale * qk * sigmoid| <= ~2), and the softmax denominator comes free from
an all-ones column appended to v.
"""
import numpy as np
import ml_dtypes

import concourse.bass as bass
import concourse.mybir as mybir
import concourse.tile as tile
from concourse import bacc
from concourse.bass_utils import run_bass_kernel_spmd

F32R = mybir.dt.float32r
F32 = mybir.dt.float32
BF16 = mybir.dt.bfloat16
AF = mybir.ActivationFunctionType
ALU = mybir.AluOpType

N_CORES = 8
DEBUG = False
STAGES = 6  # truncate build after this many stages (for profiling)
C = 512          # channels
CT = C // 128    # 4 c-tiles
N = 2048         # sequence length
NT = N // 128    # 16 m-tiles
NQ = 1024        # queries per core
H = 8            # heads
D = 64           # head dim
SCALE = D ** -0.5
EPS = 1e-5
GROUPS = 8       # 2 groups per 128-partition tile (64 ch/group)
GSIZE = (C // GROUPS) * N  # elements per group for GN stats


def build():
    nc = bacc.Bacc("TRN2", target_bir_lowering=False, debug=False,
                   num_devices=N_CORES)

    def din(name, shape, dt=F32R):
        return nc.dram_tensor(name, shape, dt, kind="ExternalInput").ap()

    peT = din("peT", [C, N])
    xT = din("xT", [C, N])
    cw1 = din("cw1", [C, C])        # conv1_w.T  [c_in, o]
    cw2 = din("cw2", [C, C])
    qw = din("qw", [C, 3 * C])      # qkv_w.T    [c_in, o]
    pw = din("pw", [C, C])          # proj_w.T
    cb1 = din("cb1", [C], F32)
    cb2 = din("cb2", [C], F32)
    gn1g = din("gn1g", [C], F32)
    gn1b = din("gn1b", [C], F32)
    gn2g = din("gn2g", [C], F32)
    gn2b = din("gn2b", [C], F32)
    pb = din("pb", [C], F32)
    gmask_in = din("gmask", [128, 2], F32)
    gmaskT_in = din("gmaskT", [2, 128], F32)
    vones_in = din("vones", [128, NT * H], BF16)
    outT = nc.dram_tensor("outT", [C, NQ], F32, kind="ExternalOutput").ap()
    dbg = {}
    if DEBUG:
        dbg["p1"] = nc.dram_tensor("dbg_p1", [128, CT, NQ], F32, kind="ExternalOutput").ap()
        dbg["p2"] = nc.dram_tensor("dbg_p2", [128, CT, N], F32, kind="ExternalOutput").ap()
        dbg["pa"] = nc.dram_tensor("dbg_pa", [128, NT, NQ], BF16, kind="ExternalOutput").ap()
        dbg["kT"] = nc.dram_tensor("dbg_kT", [128, CT, N], F32, kind="ExternalOutput").ap()
        dbg["qT"] = nc.dram_tensor("dbg_qT", [128, CT, NQ], F32, kind="ExternalOutput").ap()
        dbg["v"] = nc.dram_tensor("dbg_v", [128, NT, H, D + 1], F32, kind="ExternalOutput").ap()
        dbg["o"] = nc.dram_tensor("dbg_o", [128, CT, NQ], F32, kind="ExternalOutput").ap()

    with tile.TileContext(nc) as tc:
        _build_body(nc, tc, peT, xT, cw1, cw2, qw, pw, cb1, cb2,
                    gn1g, gn1b, gn2g, gn2b, pb, gmask_in, gmaskT_in,
                    vones_in, outT, dbg)
    nc.compile()
    return nc


def _build_body(nc, tc, peT, xT, cw1, cw2, qw, pw, cb1, cb2,
                gn1g, gn1b, gn2g, gn2b, pb, gmask_in, gmaskT_in,
                vones_in, outT, dbg):
    from contextlib import ExitStack
    ctx = ExitStack()
    with ctx:
        consts = ctx.enter_context(tc.tile_pool(name="consts", bufs=1))
        work = ctx.enter_context(tc.tile_pool(name="work", bufs=3))

        # ---- constants
        gmask = consts.tile([128, 2], F32)     # group-membership mask
        nc.sync.dma_start(gmask, gmask_in)
        gmaskT = consts.tile([2, 128], F32)
        nc.sync.dma_start(gmaskT, gmaskT_in)
        epst = consts.tile([128, 1], F32)
        nc.vector.memset(epst, EPS)
        # per-partition params: [(t p) -> p t]
        bias1 = consts.tile([128, CT], F32)
        nc.sync.dma_start(bias1, cb1.rearrange("(t p) -> p t", p=128))
        bias2 = consts.tile([128, CT], F32)
        nc.sync.dma_start(bias2, cb2.rearrange("(t p) -> p t", p=128))
        g1g = consts.tile([128, CT], F32)
        nc.sync.dma_start(g1g, gn1g.rearrange("(t p) -> p t", p=128))
        g1b = consts.tile([128, CT], F32)
        nc.sync.dma_start(g1b, gn1b.rearrange("(t p) -> p t", p=128))
        g2g = consts.tile([128, CT], F32)
        nc.sync.dma_start(g2g, gn2g.rearrange("(t p) -> p t", p=128))
        g2b = consts.tile([128, CT], F32)
        nc.sync.dma_start(g2b, gn2b.rearrange("(t p) -> p t", p=128))
        pbias = consts.tile([128, CT], F32)
        nc.sync.dma_start(pbias, pb.rearrange("(t p) -> p t", p=128))

        # ---- persistent activations
        pa_pool = ctx.enter_context(tc.tile_pool(name="pa", bufs=1))
        pa = pa_pool.tile([128, NT, NQ], BF16)     # sigmoid(pe_attn)^T tiles

        # ================= stage A/B: conv + groupnorm =================
        ps_abcd = tc.tile_pool(name="ps_mm", bufs=4, space="PSUM")
        ps_mm = ps_abcd.__enter__()
        with tc.tile_pool(name="pe_pool", bufs=1) as pe_pool, \
             tc.tile_pool(name="cw_pool", bufs=1) as cw_pool, \
             tc.tile_pool(name="p12", bufs=1) as p12_pool:
            pe_sb = pe_pool.tile([128, CT, N], F32R)
            pe_r = peT.rearrange("(t p) n -> p t n", p=128)
            for ct, eng in enumerate((nc.sync, nc.scalar, nc.gpsimd,
                                      nc.sync)):
                eng.dma_start(pe_sb[:, ct], pe_r[:, ct])
            cw1_sb = cw_pool.tile([128, CT, C], F32R)
            nc.gpsimd.dma_start(cw1_sb, cw1.rearrange("(t p) o -> p t o", p=128))
            cw2_sb = cw_pool.tile([128, CT, C], F32R)
            nc.scalar.dma_start(cw2_sb, cw2.rearrange("(t p) o -> p t o", p=128))

            # p1 only needs its first NQ columns kept; p2 needs all N.
            p1_sb = p12_pool.tile([128, CT, NQ], F32R)
            p2_sb = p12_pool.tile([128, CT, N], F32R)

            for conv_i, (cwsb, cbt, gg, gb, dst, keep) in enumerate([
                    (cw1_sb, bias1, g1g, g1b, p1_sb, NQ),
                    (cw2_sb, bias2, g2g, g2b, p2_sb, N)]):
                stats = work.tile([128, CT, N // 512, 6], F32, tag="gnstats")
                mv2 = work.tile([128, 2, CT], F32, tag="gnmv")
                stack3 = work.tile([128, 3, CT], F32, tag="gnstack")
                for ot in range(CT):
                    for nch in range(N // 512):
                        ps = ps_mm.tile([128, 512], F32, tag="mm")
                        for ct in range(CT):
                            nc.tensor.matmul(
                                ps, cwsb[:, ct, ot * 128:(ot + 1) * 128],
                                pe_sb[:, ct, nch * 512:(nch + 1) * 512],
                                start=(ct == 0), stop=(ct == CT - 1))
                        nc.vector.bn_stats(stats[:, ot, nch], ps)
                        if nch * 512 < keep:
                            nc.scalar.copy(dst[:, ot, nch * 512:(nch + 1) * 512], ps)
                    nc.vector.bn_aggr(mv2[:, :, ot], stats[:, ot])
                nc.vector.tensor_add(stack3[:, 0], mv2[:, 0], cbt)
                nc.vector.tensor_copy(stack3[:, 1], mv2[:, 1])
                nc.vector.tensor_mul(stack3[:, 2], stack3[:, 0], stack3[:, 0])
                # group sums over 64-partition halves (all ots at once, N=12)
                gs = ps_mm.tile([2, 3, CT], F32, tag="mm")
                nc.tensor.matmul(gs, gmask, stack3.rearrange("p a t -> p (a t)"),
                                 start=True, stop=True)
                gss = work.tile([2, 3, CT], F32, tag="gss")
                nc.scalar.copy(gss, gs)
                gstat = work.tile([2, 2, CT], F32, tag="gstat")  # [mean, rstd]
                nc.vector.tensor_scalar_mul(gstat[:, 0], gss[:, 0], 1.0 / 64.0)
                vt = work.tile([2, 2, CT], F32, tag="gvtmp")
                nc.vector.tensor_add(vt[:, 0], gss[:, 1], gss[:, 2])
                nc.vector.tensor_scalar_mul(vt[:, 0], vt[:, 0], 1.0 / 64.0)
                nc.vector.tensor_mul(vt[:, 1], gstat[:, 0], gstat[:, 0])
                nc.vector.tensor_sub(vt[:, 0], vt[:, 0], vt[:, 1])
                nc.scalar.activation(vt[:, 0], vt[:, 0], AF.Sqrt, bias=epst[0:2])
                nc.vector.reciprocal(gstat[:, 1], vt[:, 0])
                # broadcast group [mean, rstd] to partitions via indicator MM
                bc_ps = ps_mm.tile([128, 2, CT], F32, tag="mm")
                nc.tensor.matmul(bc_ps, gmaskT,
                                 gstat.rearrange("p a t -> p (a t)"),
                                 start=True, stop=True)
                bcst = work.tile([128, 2, CT], F32, tag="gbc")
                nc.scalar.copy(bcst, bc_ps)
                # per-channel affine: y = x*sc + sh
                sc = work.tile([128, 2, CT], F32, tag="gsc")
                nc.vector.tensor_mul(sc[:, 0], bcst[:, 1], gg)
                nc.vector.tensor_sub(sc[:, 1], cbt, bcst[:, 0])
                nc.vector.tensor_mul(sc[:, 1], sc[:, 1], sc[:, 0])
                nc.vector.tensor_add(sc[:, 1], sc[:, 1], gb)
                for ot in range(CT):
                    nc.gpsimd.tensor_scalar(
                        dst[:, ot, 0:keep], dst[:, ot, 0:keep],
                        sc[:, 0, ot:ot + 1], sc[:, 1, ot:ot + 1],
                        op0=ALU.mult, op1=ALU.add)

            # ================= stage C: pe_attn^T = sigmoid(p2^T p1) =====
            if STAGES < 2:
                return
            for mt in range(NT):
                for nq in range(NQ // 512):
                    zps = ps_mm.tile([128, 512], F32, tag="mm")
                    for ct in range(CT):
                        nc.tensor.matmul(
                            zps, p2_sb[:, ct, mt * 128:(mt + 1) * 128],
                            p1_sb[:, ct, nq * 512:(nq + 1) * 512],
                            start=(ct == 0), stop=(ct == CT - 1))
                    nc.scalar.activation(pa[:, mt, nq * 512:(nq + 1) * 512],
                                         zps, AF.Sigmoid)
            if dbg:
                nc.sync.dma_start(dbg["p1"].bitcast(F32R), p1_sb)
                nc.sync.dma_start(dbg["p2"].bitcast(F32R), p2_sb)

        # ================= stage D: qkv =================
        if STAGES < 3:
            return
        kqv_pool = ctx.enter_context(tc.tile_pool(name="kqv", bufs=1))
        kT_sb = kqv_pool.tile([128, CT, N], F32R)
        qT_sb = kqv_pool.tile([128, CT, NQ], F32R)
        v_sb = kqv_pool.tile([128, NT, H, D + 1], BF16)

        with tc.tile_pool(name="x_pool", bufs=1) as x_pool, \
             tc.tile_pool(name="qw_pool", bufs=1) as qw_pool:
            x_sb = x_pool.tile([128, CT, N], F32R)
            x_r = xT.rearrange("(t p) n -> p t n", p=128)
            qw_sb = qw_pool.tile([128, CT, 3 * C], F32R)
            qw_r = qw.rearrange("(t p) o -> p t o", p=128)
            for ct, eng in enumerate((nc.sync, nc.scalar, nc.gpsimd,
                                      nc.sync)):
                eng.dma_start(x_sb[:, ct], x_r[:, ct])
                eng.dma_start(qw_sb[:, ct], qw_r[:, ct])

            # kT (full N) and qT (first NQ)
            for ot in range(CT):
                for nch in range(N // 512):
                    ps = ps_mm.tile([128, 512], F32, tag="mm")
                    for ct in range(CT):
                        nc.tensor.matmul(
                            ps, qw_sb[:, ct, C + ot * 128:C + (ot + 1) * 128],
                            x_sb[:, ct, nch * 512:(nch + 1) * 512],
                            start=(ct == 0), stop=(ct == CT - 1))
                    nc.scalar.copy(kT_sb[:, ot, nch * 512:(nch + 1) * 512], ps)
                for nch in range(NQ // 512):
                    ps = ps_mm.tile([128, 512], F32, tag="mm")
                    for ct in range(CT):
                        nc.tensor.matmul(
                            ps, qw_sb[:, ct, ot * 128:(ot + 1) * 128],
                            x_sb[:, ct, nch * 512:(nch + 1) * 512],
                            start=(ct == 0), stop=(ct == CT - 1))
                    nc.vector.tensor_copy(qT_sb[:, ot, nch * 512:(nch + 1) * 512], ps)
            # v in natural [n, o] layout, interleaved per head with a ones col
            nc.sync.dma_start(
                v_sb[:, :, :, D:D + 1].rearrange("p t o u -> p (t o u)"),
                vones_in)
            for nt in range(NT):
                ps = ps_mm.tile([128, 512], F32, tag="mm")
                for ct in range(CT):
                    nc.tensor.matmul(
                        ps, x_sb[:, ct, nt * 128:(nt + 1) * 128],
                        qw_sb[:, ct, 2 * C:3 * C],
                        start=(ct == 0), stop=(ct == CT - 1))
                nc.vector.tensor_copy(v_sb[:, nt, :, 0:D],
                                      ps.rearrange("p (h d) -> p h d", h=H))

        if dbg:
            nc.sync.dma_start(dbg["pa"], pa)
            nc.sync.dma_start(dbg["kT"].bitcast(F32R), kT_sb)
            nc.sync.dma_start(dbg["qT"].bitcast(F32R), qT_sb)
            nc.sync.dma_start(dbg["v"].bitcast(F32R), v_sb)

        # ================= stage E: attention =================
        ps_abcd.__exit__(None, None, None)
        if STAGES < 4:
            return
        out_pool = ctx.enter_context(tc.tile_pool(name="outp", bufs=1))
        attw = ctx.enter_context(tc.tile_pool(name="attw", bufs=2))
        o_sb = out_pool.tile([128, CT, NQ], F32R)

        ps_e = ExitStack()
        ps_s = ps_e.enter_context(tc.tile_pool(name="ps_s", bufs=2,
                                               space="PSUM"))
        ps_u = ps_e.enter_context(tc.tile_pool(name="ps_u", bufs=4,
                                               space="PSUM"))

        for hp in range(H // 2):          # head pairs share a 128-row tile
            kt = hp
            for nq in range(NQ // 512):
                ua = ps_u.tile([D + 1, 512], F32, tag="u")
                ub = ps_u.tile([D + 1, 512], F32, tag="u")
                for mt2 in range(NT // 2):
                    mts = (2 * mt2, 2 * mt2 + 1)
                    sa = ps_s.tile([128, 2, 512], F32, tag="s")
                    sb_ = ps_s.tile([128, 2, 512], F32, tag="s")
                    for j, mt in enumerate(mts):
                        nc.tensor.matmul(
                            sa[:, j],
                            kT_sb[0:64, kt, mt * 128:(mt + 1) * 128],
                            qT_sb[0:64, kt, nq * 512:(nq + 1) * 512],
                            start=True, stop=True)
                        nc.tensor.matmul(
                            sb_[:, j],
                            kT_sb[64:128, kt, mt * 128:(mt + 1) * 128],
                            qT_sb[64:128, kt, nq * 512:(nq + 1) * 512],
                            start=True, stop=True)
                    t2 = attw.tile([128, 2, 2, 512], F32, tag="t2")
                    nc.vector.tensor_mul(
                        t2[:, 0], sa,
                        pa[:, 2 * mt2:2 * mt2 + 2, nq * 512:(nq + 1) * 512])
                    nc.vector.tensor_mul(
                        t2[:, 1], sb_,
                        pa[:, 2 * mt2:2 * mt2 + 2, nq * 512:(nq + 1) * 512])
                    e2 = attw.tile([128, 2, 2, 512], BF16, tag="e2")
                    nc.scalar.activation(e2, t2, AF.Exp, scale=SCALE)
                    for j, mt in enumerate(mts):
                        nc.tensor.matmul(ua, v_sb[:, mt, 2 * hp, :],
                                         e2[:, 0, j],
                                         start=(mt == 0), stop=(mt == NT - 1))
                        nc.tensor.matmul(ub, v_sb[:, mt, 2 * hp + 1, :],
                                         e2[:, 1, j],
                                         start=(mt == 0), stop=(mt == NT - 1))
                for (u, row0) in ((ua, 0), (ub, 64)):
                    rec = work.tile([1, 512], F32, tag="rec")
                    nc.vector.reciprocal(rec, u[D:D + 1])
                    bc = work.tile([64, 512], F32, tag="recbc")
                    nc.gpsimd.partition_broadcast(bc, rec)
                    nc.vector.tensor_mul(
                        o_sb[row0:row0 + 64, kt, nq * 512:(nq + 1) * 512],
                        u[0:D], bc)

        # ================= stage F: proj =================
        ps_e.close()
        if STAGES < 5:
            return
        with tc.tile_pool(name="pw_pool", bufs=1) as pw_pool, \
             tc.tile_pool(name="ps_f", bufs=2, space="PSUM") as ps_mm:
            pw_sb = pw_pool.tile([128, CT, C], F32R)
            nc.sync.dma_start(pw_sb, pw.rearrange("(t p) o -> p t o", p=128))
            fin = out_pool.tile([128, CT, NQ], F32)
            for ot in range(CT):
                for nq in range(NQ // 512):
                    ps = ps_mm.tile([128, 512], F32, tag="mm")
                    for ct in range(CT):
                        nc.tensor.matmul(
                            ps, pw_sb[:, ct, ot * 128:(ot + 1) * 128],
                            o_sb[:, ct, nq * 512:(nq + 1) * 512],
                            start=(ct == 0), stop=(ct == CT - 1))
                    nc.vector.tensor_scalar_add(
                        fin[:, ot, nq * 512:(nq + 1) * 512], ps,
                        pbias[:, ot:ot + 1])
            nc.sync.dma_start(outT.rearrange("(t p) n -> p t n", p=128), fin)


_NC_CACHE = {}


def _get_nc():
    if "nc" not in _NC_CACHE:
        _NC_CACHE["nc"] = build()
    return _NC_CACHE["nc"]


def make_in_maps(x, pe, qkv_w, proj_w, proj_b, conv1_w, conv1_b, gn1_g, gn1_b,
                 conv2_w, conv2_b, gn2_g, gn2_b):
    f = np.float32
    shared = {
        "cw1": np.ascontiguousarray(np.asarray(conv1_w, f).T),
        "cw2": np.ascontiguousarray(np.asarray(conv2_w, f).T),
        "qw": np.ascontiguousarray(np.asarray(qkv_w, f).T),
        "pw": np.ascontiguousarray(np.asarray(proj_w, f).T),
        "cb1": np.asarray(conv1_b, f),
        "cb2": np.asarray(conv2_b, f),
        "gn1g": np.asarray(gn1_g, f),
        "gn1b": np.asarray(gn1_b, f),
        "gn2g": np.asarray(gn2_g, f),
        "gn2b": np.asarray(gn2_b, f),
        "pb": np.asarray(proj_b, f),
        "gmask": np.repeat(np.eye(2, dtype=f), 64, axis=0),
        "gmaskT": np.ascontiguousarray(np.repeat(np.eye(2, dtype=f), 64, axis=0).T),
        "vones": np.ones((128, NT * H), np.float32).astype(ml_dtypes.bfloat16),
    }
    in_maps = []
    for c in range(N_CORES):
        b, h = c // 2, c % 2
        xT = np.asarray(x[b], f).T
        peT = np.asarray(pe[b], f).T
        if h == 1:
            xT = np.concatenate([xT[:, NQ:], xT[:, :NQ]], axis=1)
            peT = np.concatenate([peT[:, NQ:], peT[:, :NQ]], axis=1)
        m = dict(shared)
        m["xT"] = np.ascontiguousarray(xT)
        m["peT"] = np.ascontiguousarray(peT)
        in_maps.append(m)
    return in_maps


def assemble_out(results):
    B = N_CORES // 2
    out = np.empty((B, N, C), np.float32)
    for c in range(N_CORES):
        b, h = c // 2, c % 2
        out[b, h * NQ:(h + 1) * NQ, :] = results[c]["outT"].T
    return out


def kernel(**inputs):
    nc = _get_nc()
    in_maps = make_in_maps(**inputs)
    r = run_bass_kernel_spmd(nc, in_maps, core_ids=list(range(N_CORES)))
    return assemble_out(r.results)


if __name__ == "__main__":
    nc = build()
    print("build+compile OK")



# revision 2
# speedup vs baseline: 1.1361x; 1.1361x over previous
"""TRN2 Bass kernel for nn_Attention_87308095193383.

Sharding: 8 cores = (batch b in 0..3) x (query-half h in 0..1).
Each core computes, for its batch:
  - conv1/conv2 + GroupNorm fully (stats need full N),
  - pe_attn^T slice [m=2048, n=1024] for its query half,
  - k,v fully; q for its half,
  - attention + proj for its half -> outT [512, 1024].
Host permutes the N columns per core so "my queries" are always columns
0:1024 of the device program (SPMD: one program, per-core data).

All matmuls use float32r (1 cycle/row on TRN2 for free-dim >= 256,
~1.5e-4 rel err). Softmax skips max-subtraction (logits are O(1) here:
|scale * qk * sigmoid| <= ~2), and the softmax denominator comes free from
an all-ones column appended to v.
"""
import numpy as np
import ml_dtypes

import concourse.bass as bass
import concourse.mybir as mybir
import concourse.tile as tile
from concourse import bacc
from concourse.bass_utils import run_bass_kernel_spmd

F32R = mybir.dt.float32r
F32 = mybir.dt.float32
BF16 = mybir.dt.bfloat16
AF = mybir.ActivationFunctionType
ALU = mybir.AluOpType

N_CORES = 8
DEBUG = False
STAGES = 6  # truncate build after this many stages (for profiling)
C = 512          # channels
CT = C // 128    # 4 c-tiles
N = 2048         # sequence length
NT = N // 128    # 16 m-tiles
NQ = 1024        # queries per core
H = 8            # heads
D = 64           # head dim
SCALE = D ** -0.5
EPS = 1e-5
GROUPS = 8       # 2 groups per 128-partition tile (64 ch/group)
GSIZE = (C // GROUPS) * N  # elements per group for GN stats


def build():
    nc = bacc.Bacc("TRN2", target_bir_lowering=False, debug=False,
                   num_devices=N_CORES)

    def din(name, shape, dt=F32R):
        return nc.dram_tensor(name, shape, dt, kind="ExternalInput").ap()

    peT = din("peT", [C, N])
    xT = din("xT", [C, N])
    cw1 = din("cw1", [C, C])        # conv1_w.T  [c_in, o]
    cw2 = din("cw2", [C, C])
    qw = din("qw", [C, 3 * C])      # qkv_w.T    [c_in, o]
    pw = din("pw", [C, C])          # proj_w.T
    cb1 = din("cb1", [C], F32)
    cb2 = din("cb2", [C], F32)
    gn1g = din("gn1g", [C], F32)
    gn1b = din("gn1b", [C], F32)
    gn2g = din("gn2g", [C], F32)
    gn2b = din("gn2b", [C], F32)
    pb = din("pb", [C], F32)
    gmask_in = din("gmask", [128, 2], F32)
    gmaskT_in = din("gmaskT", [2, 128], F32)
    vones_in = din("vones", [128, NT * H], BF16)
    outT = nc.dram_tensor("outT", [C, NQ], F32, kind="ExternalOutput").ap()
    dbg = {}
    if DEBUG:
        dbg["p1"] = nc.dram_tensor("dbg_p1", [128, CT, NQ], F32, kind="ExternalOutput").ap()
        dbg["p2"] = nc.dram_tensor("dbg_p2", [128, CT, N], F32, kind="ExternalOutput").ap()
        dbg["pa"] = nc.dram_tensor("dbg_pa", [128, NT, NQ], BF16, kind="ExternalOutput").ap()
        dbg["kT"] = nc.dram_tensor("dbg_kT", [128, CT, N], F32, kind="ExternalOutput").ap()
        dbg["qT"] = nc.dram_tensor("dbg_qT", [128, CT, NQ], F32, kind="ExternalOutput").ap()
        dbg["v"] = nc.dram_tensor("dbg_v", [128, NT, H, D + 1], F32, kind="ExternalOutput").ap()
        dbg["o"] = nc.dram_tensor("dbg_o", [128, CT, NQ], F32, kind="ExternalOutput").ap()

    with tile.TileContext(nc) as tc:
        _build_body(nc, tc, peT, xT, cw1, cw2, qw, pw, cb1, cb2,
                    gn1g, gn1b, gn2g, gn2b, pb, gmask_in, gmaskT_in,
                    vones_in, outT, dbg)
    nc.compile()
    return nc


def _build_body(nc, tc, peT, xT, cw1, cw2, qw, pw, cb1, cb2,
                gn1g, gn1b, gn2g, gn2b, pb, gmask_in, gmaskT_in,
                vones_in, outT, dbg):
    from contextlib import ExitStack
    ctx = ExitStack()
    with ctx:
        consts = ctx.enter_context(tc.tile_pool(name="consts", bufs=1))
        work = ctx.enter_context(tc.tile_pool(name="work", bufs=3))

        # ---- constants
        gmask = consts.tile([128, 2], F32)     # group-membership mask
        nc.sync.dma_start(gmask, gmask_in)
        gmaskT = consts.tile([2, 128], F32)
        nc.sync.dma_start(gmaskT, gmaskT_in)
        epst = consts.tile([128, 1], F32)
        nc.vector.memset(epst, EPS)
        # per-partition params: [(t p) -> p t]
        bias1 = consts.tile([128, CT], F32)
        nc.sync.dma_start(bias1, cb1.rearrange("(t p) -> p t", p=128))
        bias2 = consts.tile([128, CT], F32)
        nc.sync.dma_start(bias2, cb2.rearrange("(t p) -> p t", p=128))
        g1g = consts.tile([128, CT], F32)
        nc.sync.dma_start(g1g, gn1g.rearrange("(t p) -> p t", p=128))
        g1b = consts.tile([128, CT], F32)
        nc.sync.dma_start(g1b, gn1b.rearrange("(t p) -> p t", p=128))
        g2g = consts.tile([128, CT], F32)
        nc.sync.dma_start(g2g, gn2g.rearrange("(t p) -> p t", p=128))
        g2b = consts.tile([128, CT], F32)
        nc.sync.dma_start(g2b, gn2b.rearrange("(t p) -> p t", p=128))
        pbias = consts.tile([128, CT], F32)
        nc.sync.dma_start(pbias, pb.rearrange("(t p) -> p t", p=128))

        # ---- persistent activations
        pa_pool = ctx.enter_context(tc.tile_pool(name="pa", bufs=1))
        pa = pa_pool.tile([128, NT, NQ], BF16)     # sigmoid(pe_attn)^T tiles

        # ================= stage A/B: conv + groupnorm =================
        ps_abcd = tc.tile_pool(name="ps_mm", bufs=4, space="PSUM")
        ps_mm = ps_abcd.__enter__()
        with tc.tile_pool(name="pe_pool", bufs=1) as pe_pool, \
             tc.tile_pool(name="cw_pool", bufs=1) as cw_pool, \
             tc.tile_pool(name="p12", bufs=1) as p12_pool:
            pe_sb = pe_pool.tile([128, CT, N], F32R)
            pe_r = peT.rearrange("(t p) n -> p t n", p=128)
            for ct, eng in enumerate((nc.sync, nc.scalar, nc.gpsimd,
                                      nc.sync)):
                eng.dma_start(pe_sb[:, ct], pe_r[:, ct])
            cw1_sb = cw_pool.tile([128, CT, C], F32R)
            nc.gpsimd.dma_start(cw1_sb, cw1.rearrange("(t p) o -> p t o", p=128))
            cw2_sb = cw_pool.tile([128, CT, C], F32R)
            nc.scalar.dma_start(cw2_sb, cw2.rearrange("(t p) o -> p t o", p=128))

            # p1 only needs its first NQ columns kept; p2 needs all N.
            p1_sb = p12_pool.tile([128, CT, NQ], F32R)
            p2_sb = p12_pool.tile([128, CT, N], F32R)

            for conv_i, (cwsb, cbt, gg, gb, dst, keep) in enumerate([
                    (cw1_sb, bias1, g1g, g1b, p1_sb, NQ),
                    (cw2_sb, bias2, g2g, g2b, p2_sb, N)]):
                stats = work.tile([128, CT, N // 512, 6], F32, tag="gnstats")
                mv2 = work.tile([128, 2, CT], F32, tag="gnmv")
                stack3 = work.tile([128, 3, CT], F32, tag="gnstack")
                for ot in range(CT):
                    for nch in range(N // 512):
                        ps = ps_mm.tile([128, 512], F32, tag="mm")
                        for ct in range(CT):
                            nc.tensor.matmul(
                                ps, cwsb[:, ct, ot * 128:(ot + 1) * 128],
                                pe_sb[:, ct, nch * 512:(nch + 1) * 512],
                                start=(ct == 0), stop=(ct == CT - 1))
                        nc.vector.bn_stats(stats[:, ot, nch], ps)
                        if nch * 512 < keep:
                            nc.scalar.copy(dst[:, ot, nch * 512:(nch + 1) * 512], ps)
                    nc.vector.bn_aggr(mv2[:, :, ot], stats[:, ot])
                nc.vector.tensor_add(stack3[:, 0], mv2[:, 0], cbt)
                nc.vector.tensor_copy(stack3[:, 1], mv2[:, 1])
                nc.vector.tensor_mul(stack3[:, 2], stack3[:, 0], stack3[:, 0])
                # group sums over 64-partition halves (all ots at once, N=12)
                gs = ps_mm.tile([2, 3, CT], F32, tag="mm")
                nc.tensor.matmul(gs, gmask, stack3.rearrange("p a t -> p (a t)"),
                                 start=True, stop=True)
                gss = work.tile([2, 3, CT], F32, tag="gss")
                nc.scalar.copy(gss, gs)
                gstat = work.tile([2, 2, CT], F32, tag="gstat")  # [mean, rstd]
                nc.vector.tensor_scalar_mul(gstat[:, 0], gss[:, 0], 1.0 / 64.0)
                vt = work.tile([2, 2, CT], F32, tag="gvtmp")
                nc.vector.tensor_add(vt[:, 0], gss[:, 1], gss[:, 2])
                nc.vector.tensor_scalar_mul(vt[:, 0], vt[:, 0], 1.0 / 64.0)
                nc.vector.tensor_mul(vt[:, 1], gstat[:, 0], gstat[:, 0])
                nc.vector.tensor_sub(vt[:, 0], vt[:, 0], vt[:, 1])
                nc.scalar.activation(vt[:, 0], vt[:, 0], AF.Sqrt, bias=epst[0:2])
                nc.vector.reciprocal(gstat[:, 1], vt[:, 0])
                # broadcast group [mean, rstd] to partitions via indicator MM
                bc_ps = ps_mm.tile([128, 2, CT], F32, tag="mm")
                nc.tensor.matmul(bc_ps, gmaskT,
                                 gstat.rearrange("p a t -> p (a t)"),
                                 start=True, stop=True)
                bcst = work.tile([128, 2, CT], F32, tag="gbc")
                nc.scalar.copy(bcst, bc_ps)
                # per-channel affine: y = x*sc + sh
                sc = work.tile([128, 2, CT], F32, tag="gsc")
                nc.vector.tensor_mul(sc[:, 0], bcst[:, 1], gg)
                nc.vector.tensor_sub(sc[:, 1], cbt, bcst[:, 0])
                nc.vector.tensor_mul(sc[:, 1], sc[:, 1], sc[:, 0])
                nc.vector.tensor_add(sc[:, 1], sc[:, 1], gb)
                for ot in range(CT):
                    nc.gpsimd.tensor_scalar(
                        dst[:, ot, 0:keep], dst[:, ot, 0:keep],
                        sc[:, 0, ot:ot + 1], sc[:, 1, ot:ot + 1],
                        op0=ALU.mult, op1=ALU.add)

            # ================= stage C: pe_attn^T = sigmoid(p2^T p1) =====
            if STAGES < 2:
                return
            for mt in range(NT):
                for nq in range(NQ // 512):
                    zps = ps_mm.tile([128, 512], F32, tag="mm")
                    for ct in range(CT):
                        nc.tensor.matmul(
                            zps, p2_sb[:, ct, mt * 128:(mt + 1) * 128],
                            p1_sb[:, ct, nq * 512:(nq + 1) * 512],
                            start=(ct == 0), stop=(ct == CT - 1))
                    nc.scalar.activation(pa[:, mt, nq * 512:(nq + 1) * 512],
                                         zps, AF.Sigmoid)
            if dbg:
                nc.sync.dma_start(dbg["p1"].bitcast(F32R), p1_sb)
                nc.sync.dma_start(dbg["p2"].bitcast(F32R), p2_sb)

        # ================= stage D: qkv =================
        if STAGES < 3:
            return
        kqv_pool = ctx.enter_context(tc.tile_pool(name="kqv", bufs=1))
        kT_sb = kqv_pool.tile([128, CT, N], F32R)
        qT_sb = kqv_pool.tile([128, CT, NQ], F32R)
        v_sb = kqv_pool.tile([128, NT, H, D + 1], BF16)

        with tc.tile_pool(name="x_pool", bufs=1) as x_pool, \
             tc.tile_pool(name="qw_pool", bufs=1) as qw_pool:
            x_sb = x_pool.tile([128, CT, N], F32R)
            x_r = xT.rearrange("(t p) n -> p t n", p=128)
            qw_sb = qw_pool.tile([128, CT, 3 * C], F32R)
            qw_r = qw.rearrange("(t p) o -> p t o", p=128)
            for ct, eng in enumerate((nc.sync, nc.scalar, nc.gpsimd,
                                      nc.sync)):
                eng.dma_start(x_sb[:, ct], x_r[:, ct])
                eng.dma_start(qw_sb[:, ct], qw_r[:, ct])

            # kT (full N) and qT (first NQ)
            for ot in range(CT):
                for nch in range(N // 512):
                    ps = ps_mm.tile([128, 512], F32, tag="mm")
                    for ct in range(CT):
                        nc.tensor.matmul(
                            ps, qw_sb[:, ct, C + ot * 128:C + (ot + 1) * 128],
                            x_sb[:, ct, nch * 512:(nch + 1) * 512],
                            start=(ct == 0), stop=(ct == CT - 1))
                    nc.scalar.copy(kT_sb[:, ot, nch * 512:(nch + 1) * 512], ps)
                for nch in range(NQ // 512):
                    ps = ps_mm.tile([128, 512], F32, tag="mm")
                    for ct in range(CT):
                        nc.tensor.matmul(
                            ps, qw_sb[:, ct, ot * 128:(ot + 1) * 128],
                            x_sb[:, ct, nch * 512:(nch + 1) * 512],
                            start=(ct == 0), stop=(ct == CT - 1))
                    nc.vector.tensor_copy(qT_sb[:, ot, nch * 512:(nch + 1) * 512], ps)
            # v in natural [n, o] layout, interleaved per head with a ones col
            nc.sync.dma_start(
                v_sb[:, :, :, D:D + 1].rearrange("p t o u -> p (t o u)"),
                vones_in)
            for nt in range(NT):
                ps = ps_mm.tile([128, 512], F32, tag="mm")
                for ct in range(CT):
                    nc.tensor.matmul(
                        ps, x_sb[:, ct, nt * 128:(nt + 1) * 128],
                        qw_sb[:, ct, 2 * C:3 * C],
                        start=(ct == 0), stop=(ct == CT - 1))
                nc.vector.tensor_copy(v_sb[:, nt, :, 0:D],
                                      ps.rearrange("p (h d) -> p h d", h=H))

        if dbg:
            nc.sync.dma_start(dbg["pa"], pa)
            nc.sync.dma_start(dbg["kT"].bitcast(F32R), kT_sb)
            nc.sync.dma_start(dbg["qT"].bitcast(F32R), qT_sb)
            nc.sync.dma_start(dbg["v"].bitcast(F32R), v_sb)

        # ================= stage E: attention =================
        ps_abcd.__exit__(None, None, None)
        if STAGES < 4:
            return
        out_pool = ctx.enter_context(tc.tile_pool(name="outp", bufs=1))
        attw = ctx.enter_context(tc.tile_pool(name="attw", bufs=2))
        o_sb = out_pool.tile([128, CT, NQ], F32R)

        ps_e = ExitStack()
        ps_s = ps_e.enter_context(tc.tile_pool(name="ps_s", bufs=2,
                                               space="PSUM"))
        ps_u = ps_e.enter_context(tc.tile_pool(name="ps_u", bufs=4,
                                               space="PSUM"))

        for hp in range(H // 2):          # head pairs share a 128-row tile
            kt = hp
            for nq in range(NQ // 512):
                ua = ps_u.tile([D + 1, 512], F32, tag="u")
                ub = ps_u.tile([D + 1, 512], F32, tag="u")
                for mt2 in range(NT // 2):
                    mts = (2 * mt2, 2 * mt2 + 1)
                    sa = ps_s.tile([128, 2, 512], F32, tag="s")
                    sb_ = ps_s.tile([128, 2, 512], F32, tag="s")
                    for j, mt in enumerate(mts):
                        nc.tensor.matmul(
                            sa[:, j],
                            kT_sb[0:64, kt, mt * 128:(mt + 1) * 128],
                            qT_sb[0:64, kt, nq * 512:(nq + 1) * 512],
                            start=True, stop=True)
                        nc.tensor.matmul(
                            sb_[:, j],
                            kT_sb[64:128, kt, mt * 128:(mt + 1) * 128],
                            qT_sb[64:128, kt, nq * 512:(nq + 1) * 512],
                            start=True, stop=True)
                    t2 = attw.tile([128, 2, 2, 512], F32, tag="t2")
                    nc.vector.tensor_mul(
                        t2[:, 0], sa,
                        pa[:, 2 * mt2:2 * mt2 + 2, nq * 512:(nq + 1) * 512])
                    nc.vector.tensor_mul(
                        t2[:, 1], sb_,
                        pa[:, 2 * mt2:2 * mt2 + 2, nq * 512:(nq + 1) * 512])
                    e2 = attw.tile([128, 2, 2, 512], BF16, tag="e2")
                    nc.scalar.activation(e2, t2, AF.Exp, scale=SCALE)
                    for j, mt in enumerate(mts):
                        nc.tensor.matmul(ua, v_sb[:, mt, 2 * hp, :],
                                         e2[:, 0, j],
                                         start=(mt == 0), stop=(mt == NT - 1))
                        nc.tensor.matmul(ub, v_sb[:, mt, 2 * hp + 1, :],
                                         e2[:, 1, j],
                                         start=(mt == 0), stop=(mt == NT - 1))
                for (u, row0) in ((ua, 0), (ub, 64)):
                    rec = work.tile([1, 512], F32, tag="rec")
                    nc.vector.reciprocal(rec, u[D:D + 1])
                    bc = work.tile([64, 512], F32, tag="recbc")
                    nc.gpsimd.partition_broadcast(bc, rec)
                    nc.vector.tensor_mul(
                        o_sb[row0:row0 + 64, kt, nq * 512:(nq + 1) * 512],
                        u[0:D], bc)

        # ================= stage F: proj =================
        ps_e.close()
        if STAGES < 5:
            return
        with tc.tile_pool(name="pw_pool", bufs=1) as pw_pool, \
             tc.tile_pool(name="ps_f", bufs=2, space="PSUM") as ps_mm:
            pw_sb = pw_pool.tile([128, CT, C], F32R)
            nc.sync.dma_start(pw_sb, pw.rearrange("(t p) o -> p t o", p=128))
            fin = out_pool.tile([128, CT, NQ], F32)
            for ot in range(CT):
                for nq in range(NQ // 512):
                    ps = ps_mm.tile([128, 512], F32, tag="mm")
                    for ct in range(CT):
                        nc.tensor.matmul(
                            ps, pw_sb[:, ct, ot * 128:(ot + 1) * 128],
                            o_sb[:, ct, nq * 512:(nq + 1) * 512],
                            start=(ct == 0), stop=(ct == CT - 1))
                    nc.vector.tensor_scalar_add(
                        fin[:, ot, nq * 512:(nq + 1) * 512], ps,
                        pbias[:, ot:ot + 1])
            nc.sync.dma_start(outT.rearrange("(t p) n -> p t n", p=128), fin)


_NC_CACHE = {}


def _get_nc():
    if "nc" not in _NC_CACHE:
        _NC_CACHE["nc"] = build()
    return _NC_CACHE["nc"]


def make_in_maps(x, pe, qkv_w, proj_w, proj_b, conv1_w, conv1_b, gn1_g, gn1_b,
                 conv2_w, conv2_b, gn2_g, gn2_b):
    f = np.float32
    shared = {
        "cw1": np.ascontiguousarray(np.asarray(conv1_w, f).T),
        "cw2": np.ascontiguousarray(np.asarray(conv2_w, f).T),
        "qw": np.ascontiguousarray(np.asarray(qkv_w, f).T),
        "pw": np.ascontiguousarray(np.asarray(proj_w, f).T),
        "cb1": np.asarray(conv1_b, f),
        "cb2": np.asarray(conv2_b, f),
        "gn1g": np.asarray(gn1_g, f),
        "gn1b": np.asarray(gn1_b, f),
        "gn2g": np.asarray(gn2_g, f),
        "gn2b": np.asarray(gn2_b, f),
        "pb": np.asarray(proj_b, f),
        "gmask": np.repeat(np.eye(2, dtype=f), 64, axis=0),
        "gmaskT": np.ascontiguousarray(np.repeat(np.eye(2, dtype=f), 64, axis=0).T),
        "vones": np.ones((128, NT * H), np.float32).astype(ml_dtypes.bfloat16),
    }
    in_maps = []
    for c in range(N_CORES):
        b, h = c // 2, c % 2
        xT = np.asarray(x[b], f).T
        peT = np.asarray(pe[b], f).T
        if h == 1:
            xT = np.concatenate([xT[:, NQ:], xT[:, :NQ]], axis=1)
            peT = np.concatenate([peT[:, NQ:], peT[:, :NQ]], axis=1)
        m = dict(shared)
        m["xT"] = np.ascontiguousarray(xT)
        m["peT"] = np.ascontiguousarray(peT)
        in_maps.append(m)
    return in_maps


def assemble_out(results):
    B = N_CORES // 2
    out = np.empty((B, N, C), np.float32)
    for c in range(N_CORES):
        b, h = c // 2, c % 2
        out[b, h * NQ:(h + 1) * NQ, :] = results[c]["outT"].T
    return out


PROFILE = False
LAST_RESULT = None


def kernel(**inputs):
    global LAST_RESULT
    nc = _get_nc()
    in_maps = make_in_maps(**inputs)
    r = run_bass_kernel_spmd(nc, in_maps, core_ids=list(range(N_CORES)),
                             trace=PROFILE)
    LAST_RESULT = r
    return assemble_out(r.results)


if __name__ == "__main__":
    nc = build()
    print("build+compile OK")

